# revision 9
# baseline (speedup 1.0000x reference)
import sys

sys.path.insert(0, "/opt/trn_rl_repo")

import numpy as np
import ml_dtypes
from contextlib import ExitStack

import concourse.bass as bass
import concourse.tile as tile
from concourse import bacc
from concourse import mybir
from concourse.bass_utils import run_bass_kernel_spmd

B, S, HID = 2, 2048, 1024
NH, NKV, HD = 16, 4, 64
P = 128
NK = HID // P
NQC = S // 512
NST = S // P
QH = NH // NKV
FEAT = QH * HD
MASK_NEG = -1e9

F32 = mybir.dt.float32
F32R = mybir.dt.float32r
BF16 = mybir.dt.bfloat16


def _r(ap):
    return ap.bitcast(F32R)


def _pin_act_tables():
    import concourse.hw_specs as hw_specs
    import concourse.bacc as bacc_mod
    real = hw_specs.get_activation_tables

    def pinned(arch):
        tabs = dict(real(arch))
        return {name: (funcs if name == "natural_log_exp_and_others" else set())
                for name, funcs in tabs.items()}

    bacc_mod.get_activation_tables = pinned


def build_program():
    _pin_act_tables()
    nc = bacc.Bacc("TRN2", target_bir_lowering=False, debug=False)

    d_xT = nc.dram_tensor("xT", [HID, S], BF16, kind="ExternalInput").ap()
    d_wqT = nc.dram_tensor("wqT", [HID, FEAT], BF16, kind="ExternalInput").ap()
    d_wkvT = nc.dram_tensor("wkvT", [HID, 2 * HD], BF16,
                            kind="ExternalInput").ap()
    d_woT = nc.dram_tensor("woT", [FEAT, HID], BF16, kind="ExternalInput").ap()
    d_cosT = nc.dram_tensor("cosT", [P, S], BF16, kind="ExternalInput").ap()
    d_sinT = nc.dram_tensor("sinT", [P, S], BF16, kind="ExternalInput").ap()
    d_r128 = nc.dram_tensor("r128", [P, P], BF16, kind="ExternalInput").ap()
    d_rdup = nc.dram_tensor("rdup", [HD, P], BF16, kind="ExternalInput").ap()
    d_idup = nc.dram_tensor("idup", [HD, P], BF16, kind="ExternalInput").ap()
    d_ident = nc.dram_tensor("ident", [P, HD], BF16, kind="ExternalInput").ap()
    d_ishift = nc.dram_tensor("ishift", [HD, P], BF16,
                              kind="ExternalInput").ap()
    d_tri = nc.dram_tensor("tri", [P, P], BF16, kind="ExternalInput").ap()
    d_ones1 = nc.dram_tensor("ones1c", [1, HD], F32R, kind="ExternalInput").ap()
    d_onesv = nc.dram_tensor("onesv", [P, NST], BF16, kind="ExternalInput").ap()
    d_i128b = nc.dram_tensor("i128b", [P, P], BF16, kind="ExternalInput").ap()
    d_out = nc.dram_tensor("outp", [S, HID], BF16, kind="ExternalOutput").ap()

    with tile.TileContext(nc) as tc, ExitStack() as ctx, \
            nc.allow_low_precision(reason="bf16 compute fits 2e-2 tolerance"):
        consts = ctx.enter_context(tc.tile_pool(name="consts", bufs=1))
        main = ctx.enter_context(tc.tile_pool(name="main", bufs=1))

        wq_sb = consts.tile([P, NK, FEAT], BF16)
        nc.scalar.dma_start(wq_sb[:], d_wqT.rearrange("(ko p) m -> p ko m", p=P))
        wkv_sb = consts.tile([P, NK, 2 * HD], BF16)
        nc.scalar.dma_start(wkv_sb[:],
                            d_wkvT.rearrange("(ko p) m -> p ko m", p=P))
        wo_sb = consts.tile([P, 2, HID], BF16)
        nc.gpsimd.dma_start(wo_sb[:], d_woT.rearrange("(ko p) m -> p ko m", p=P))
        cos_sb = consts.tile([P, S], BF16)
        nc.scalar.dma_start(cos_sb[:], d_cosT)
        sin_sb = consts.tile([P, S], BF16)
        nc.gpsimd.dma_start(sin_sb[:], d_sinT)
        r128_sb = consts.tile([P, P], BF16)
        nc.sync.dma_start(r128_sb[:], d_r128)
        rdup_sb = consts.tile([HD, P], BF16)
        nc.sync.dma_start(rdup_sb[:], d_rdup)
        idup_sb = consts.tile([HD, P], BF16)
        nc.sync.dma_start(idup_sb[:], d_idup)
        ident_sb = consts.tile([P, HD], BF16)
        nc.sync.dma_start(ident_sb[:], d_ident)
        ishift_sb = consts.tile([HD, P], BF16)
        nc.sync.dma_start(ishift_sb[:], d_ishift)
        tri_sb = consts.tile([P, P], BF16)
        nc.sync.dma_start(tri_sb[:], d_tri)
        i128b_sb = consts.tile([P, P], BF16)
        nc.sync.dma_start(i128b_sb[:], d_i128b)
        ones1 = consts.tile([1, HD], F32R)
        nc.sync.dma_start(ones1[:], d_ones1)

        xsb = main.tile([P, NK, S], BF16)
        dma_engs = [nc.sync, nc.gpsimd, nc.scalar]
        for n in range(NQC):
            for k in range(NK):
                eng = dma_engs[(n * NK + k) % 3]
                eng.dma_start(xsb[:, k, n * 512:(n + 1) * 512],
                              d_xT[k * P:(k + 1) * P, n * 512:(n + 1) * 512])

        qpt = main.tile([P, 2, S], BF16)
        kpt = main.tile([P, S], BF16)
        vaug = main.tile([P, NST, HD + 1], BF16)
        attnT = main.tile([P, 2, S], BF16)

        nc.sync.dma_start(vaug[:, :, HD:HD + 1], d_onesv)

        scps = ctx.enter_context(tc.tile_pool(name="scps", bufs=2, space="PSUM"))
        pvps = ctx.enter_context(tc.tile_pool(name="pvps", bufs=1, space="PSUM"))
        util = ctx.enter_context(tc.tile_pool(name="util", bufs=2, space="PSUM"))

        rawp = ctx.enter_context(tc.tile_pool(name="rawp", bufs=3))
        tmpp = ctx.enter_context(tc.tile_pool(name="tmpp", bufs=2))
        ptp = ctx.enter_context(tc.tile_pool(name="ptp", bufs=4))
        recp = ctx.enter_context(tc.tile_pool(name="recp", bufs=2))
        outp = ctx.enter_context(tc.tile_pool(name="outp", bufs=3))

        for i in range(16):
            wp = util.tile([P, P], F32, tag="ut", name=f"warm{i}")
            nc.tensor.matmul(wp[:], i128b_sb[:], i128b_sb[:],
                             start=True, stop=True)


        def emit_proj_q(n, m):
            c0 = n * 512
            ps = util.tile([P, 512], F32, tag="ut", name=f"pjq{n}{m}")
            for k in range(NK):
                nc.tensor.matmul(ps[:], wq_sb[:, k, m * P:(m + 1) * P],
                                 xsb[:, k, c0:c0 + 512],
                                 start=(k == 0), stop=(k == NK - 1))
            raw = rawp.tile([P, 512], BF16, tag="raw", name=f"qraw{n}{m}")
            nc.vector.tensor_copy(raw[:], ps[:])
            psr = util.tile([P, 512], F32, tag="ut", name=f"pjqr{n}{m}")
            nc.tensor.matmul(psr[:], r128_sb[:], raw[:], start=True, stop=True)
            cs = cos_sb[:, c0:c0 + 512]
            sn = sin_sb[:, c0:c0 + 512]
            t1 = tmpp.tile([P, 512], BF16, tag="t1", name=f"t1q{n}{m}")
            nc.vector.tensor_mul(t1[:], raw[:], cs)
            t2 = tmpp.tile([P, 512], BF16, tag="t2", name=f"t2q{n}{m}")
            nc.vector.tensor_mul(t2[:], psr[:], sn)
            nc.gpsimd.tensor_add(qpt[:, m, c0:c0 + 512], t1[:], t2[:])

        def emit_proj_kv(n):
            c0 = n * 512
            ps = util.tile([P, 512], F32, tag="ut", name=f"pjkv{n}")
            for k in range(NK):
                nc.tensor.matmul(ps[:], wkv_sb[:, k, :],
                                 xsb[:, k, c0:c0 + 512],
                                 start=(k == 0), stop=(k == NK - 1))
            raw = rawp.tile([P, 512], BF16, tag="raw", name=f"kvraw{n}")
            nc.vector.tensor_copy(raw[:], ps[:])
            cs = cos_sb[:, c0:c0 + 512]
            sn = sin_sb[:, c0:c0 + 512]
            psk2 = util.tile([P, 512], F32, tag="ut", name=f"pjk2{n}")
            nc.tensor.matmul(psk2[:], idup_sb[:], raw[0:HD, :],
                             start=True, stop=True)
            t1 = tmpp.tile([P, 512], BF16, tag="t1", name=f"t1k{n}")
            nc.vector.tensor_mul(t1[:], psk2[:], cs)
            pskr = util.tile([P, 512], F32, tag="ut", name=f"pjkr{n}")
            nc.tensor.matmul(pskr[:], rdup_sb[:], raw[0:HD, :],
                             start=True, stop=True)
            t2 = tmpp.tile([P, 512], BF16, tag="t2", name=f"t2k{n}")
            nc.vector.tensor_mul(t2[:], pskr[:], sn)
            nc.gpsimd.tensor_add(kpt[:, c0:c0 + 512], t1[:], t2[:])
            for tt in range(4):
                st = 4 * n + tt
                psv = util.tile([P, HD], BF16, tag="ut", name=f"vt{n}{tt}")
                nc.tensor.transpose(psv[:], raw[HD:P, tt * P:(tt + 1) * P],
                                    ident_sb[HD:P, :])
                nc.vector.tensor_copy(vaug[:, st, 0:HD], psv[:])

        def emit_outproj(st, nn):
            po = util.tile([P, 512], F32, tag="ut", name=f"po{st}{nn}")
            for m in range(2):
                nc.tensor.matmul(po[:], attnT[:, m, st * P:(st + 1) * P],
                                 wo_sb[:, m, nn * 512:(nn + 1) * 512],
                                 start=(m == 0), stop=(m == 1))
            ot = outp.tile([P, 512], BF16, tag="ot", name=f"ot{st}{nn}")
            nc.vector.tensor_copy(ot[:], po[:])
            dma_engs[(st * 2 + nn) % 3].dma_start(
                d_out[st * P:(st + 1) * P, nn * 512:(nn + 1) * 512], ot[:])

        fillers = []

        def pop_filler():
            pass

        def drain_until(name):
            while fillers:
                nm, fn = fillers.pop(0)
                fn()
                if nm == name:
                    return

        def emit_attn(m, j):
            c0 = j * 512
            T = 4 * j + 4
            pv = pvps.tile([HD + 1, 2, 512], F32, tag="pv", name=f"pv{m}{j}")
            for t in range(T):
                r = t - 4 * j
                lo = P * r if r >= 0 else 0
                sc = scps.tile([P, 2, 512], F32, tag="sc", name=f"sc{m}{j}{t}")
                pt = ptp.tile([P, 2, 512], BF16, tag="pt", name=f"pt{m}{j}{t}")
                for h2 in (0, 1):
                    half = h2 * HD
                    kl = kpt[half:half + HD, t * P:(t + 1) * P]
                    ql = qpt[half:half + HD, m, c0 + lo:c0 + 512]
                    if r >= 0:
                        nc.tensor.matmul(sc[:, h2, lo:512], kl, ql,
                                         start=True, stop=False,
                                         skip_group_check=True)
                        nc.tensor.matmul(sc[:, h2, lo:lo + P], i128b_sb[:],
                                         tri_sb[:], start=False, stop=True,
                                         skip_group_check=True)
                    else:
                        nc.tensor.matmul(sc[:, h2, :], kl, ql,
                                         start=True, stop=True,
                                         skip_group_check=True)
                if lo == 0:
                    nc.scalar.activation(
                        pt[:], sc[:],
                        mybir.ActivationFunctionType.Exp, scale=0.125)
                else:
                    for h2 in (0, 1):
                        nc.scalar.activation(
                            pt[:, h2, lo:512], sc[:, h2, lo:512],
                            mybir.ActivationFunctionType.Exp, scale=0.125)
                for h2 in (0, 1):
                    nc.tensor.matmul(pv[0:HD + 1, h2, lo:512], vaug[:, t, :],
                                     pt[:, h2, lo:512],
                                     start=(t == 0), stop=(t == T - 1),
                                     skip_group_check=True)
                pop_filler()
            lnt = recp.tile([1, 2, 512], F32, tag="lnt", name=f"lnt{m}{j}")
            nc.scalar.activation(lnt[:], pv[HD:HD + 1, :, :],
                                 mybir.ActivationFunctionType.Ln)
            recr = recp.tile([1, 2, 512], F32R, tag="recr", name=f"recr{m}{j}")
            nc.scalar.activation(recr[:], lnt[:],
                                 mybir.ActivationFunctionType.Exp, scale=-1.0)
            for h2 in (0, 1):
                rec_b = util.tile([HD, 512], F32, tag="ut", name=f"rb{m}{j}{h2}")
                nc.tensor.matmul(rec_b[:], ones1[:], recr[:, h2, :],
                                 start=True, stop=True)
                rec_s = recp.tile([HD, 512], BF16, tag="recs",
                                  name=f"rcs{m}{j}{h2}")
                nc.vector.tensor_copy(rec_s[:], rec_b[:])
                if h2 == 0:
                    nc.vector.tensor_mul(attnT[0:HD, m, c0:c0 + 512],
                                         pv[0:HD, 0, :], rec_s[:])
                else:
                    oddt = recp.tile([HD, 512], BF16, tag="oddt",
                                     name=f"odd{m}{j}")
                    nc.vector.tensor_mul(oddt[:], pv[0:HD, 1, :], rec_s[:])
                    rp = util.tile([P, 512], F32, tag="ut", name=f"rp{m}{j}")
                    nc.tensor.matmul(rp[:], ishift_sb[:], oddt[:],
                                     start=True, stop=True)
                    nc.vector.tensor_copy(attnT[HD:P, m, c0:c0 + 512],
                                          rp[HD:P, :])

        emit_proj_q(0, 0)
        emit_proj_kv(0)
        fillers.extend([
            ("q01", lambda: emit_proj_q(0, 1)),
            ("q10", lambda: emit_proj_q(1, 0)),
            ("kv1", lambda: emit_proj_kv(1)),
            ("q11", lambda: emit_proj_q(1, 1)),
            ("q20", lambda: emit_proj_q(2, 0)),
            ("kv2", lambda: emit_proj_kv(2)),
            ("q21", lambda: emit_proj_q(2, 1)),
            ("q30", lambda: emit_proj_q(3, 0)),
            ("kv3", lambda: emit_proj_kv(3)),
            ("q31", lambda: emit_proj_q(3, 1)),
        ])

        def add_outproj(st_lo, st_hi):
            for st in range(st_lo, st_hi):
                for nn in range(2):
                    fillers.append(
                        (f"op{st}{nn}",
                         lambda st=st, nn=nn: emit_outproj(st, nn)))

        emit_attn(0, 0)
        drain_until("q01")
        emit_attn(1, 0)
        add_outproj(0, 4)
        drain_until("kv1")
        emit_attn(0, 1)
        drain_until("q11")
        emit_attn(1, 1)
        add_outproj(4, 8)
        drain_until("kv2")
        emit_attn(0, 2)
        drain_until("q21")
        emit_attn(1, 2)
        add_outproj(8, 12)
        drain_until("kv3")
        emit_attn(0, 3)
        drain_until("q31")
        emit_attn(1, 3)
        add_outproj(12, NST)
        while fillers:
            fillers.pop(0)[1]()

    nc.compile()
    return nc


def make_consts():
    bf = ml_dtypes.bfloat16
    r128 = np.zeros((P, P), np.float32)
    for mm in range(P):
        hh, dd = mm // HD, mm % HD
        if dd < HD // 2:
            r128[hh * HD + dd + HD // 2, mm] = -1.0
        else:
            r128[hh * HD + dd - HD // 2, mm] = 1.0
    rdup = np.zeros((HD, P), np.float32)
    idup = np.zeros((HD, P), np.float32)
    for mm in range(P):
        dd = mm % HD
        idup[dd, mm] = 1.0
        if dd < HD // 2:
            rdup[dd + HD // 2, mm] = -1.0
        else:
            rdup[dd - HD // 2, mm] = 1.0
    ident = np.zeros((P, HD), np.float32)
    ident[HD:P, :] = np.eye(HD)
    ishift = np.zeros((HD, P), np.float32)
    for kk in range(HD):
        ishift[kk, kk + HD] = 1.0
    tri = np.where(np.arange(P)[:, None] <= np.arange(P)[None, :], 0.0,
                   MASK_NEG).astype(bf)
    i128b = np.eye(P).astype(bf)
    return dict(r128=r128.astype(bf), rdup=rdup.astype(bf),
                idup=idup.astype(bf), ident=ident.astype(bf),
                ishift=ishift.astype(bf), tri=tri, i128b=i128b,
                ones1c=np.ones((1, HD), np.float32),
                onesv=np.ones((P, NST), bf))


def make_in_maps(x, cos, sin, wq, wk, wv, wo):
    bf = ml_dtypes.bfloat16
    consts = make_consts()
    cosT = np.ascontiguousarray(np.vstack([cos.T, cos.T])).astype(bf)
    sinT = np.ascontiguousarray(np.vstack([sin.T, sin.T])).astype(bf)
    in_maps = []
    for core in range(8):
        b, g = core // NKV, core % NKV
        xT = np.ascontiguousarray(x[b].T).astype(bf)
        wqT = np.ascontiguousarray(wq[g * FEAT:(g + 1) * FEAT, :].T).astype(bf)
        wkvT = np.ascontiguousarray(
            np.concatenate([wk[g * HD:(g + 1) * HD, :],
                            wv[g * HD:(g + 1) * HD, :]], axis=0).T).astype(bf)
        woT = np.ascontiguousarray(wo[:, g * FEAT:(g + 1) * FEAT].T).astype(bf)
        in_maps.append(dict(xT=xT, wqT=wqT, wkvT=wkvT, woT=woT,
                            cosT=cosT, sinT=sinT, **consts))
    return in_maps


_PROG = None


def kernel(x, cos, sin, wq, wk, wv, wo):
    global _PROG
    x = np.asarray(x, np.float32)
    cos = np.asarray(cos, np.float32)
    sin = np.asarray(sin, np.float32)
    wq = np.asarray(wq, np.float32)
    wk = np.asarray(wk, np.float32)
    wv = np.asarray(wv, np.float32)
    wo = np.asarray(wo, np.float32)

    in_maps = make_in_maps(x, cos, sin, wq, wk, wv, wo)
    if _PROG is None:
        _PROG = build_program()
    res = run_bass_kernel_spmd(_PROG, in_maps, core_ids=list(range(8)))

    out = np.zeros((B, S, HID), np.float32)
    for core in range(8):
        out[core // NKV] += np.asarray(res.results[core]["outp"], np.float32)
    return out


if __name__ == "__main__":
    rng = np.random.default_rng(0)
    ins = dict(
        x=rng.standard_normal((B, S, HID)).astype(np.float32),
        cos=rng.random((S, HD)).astype(np.float32),
        sin=rng.random((S, HD)).astype(np.float32),
        wq=(rng.standard_normal((HID, HID)) * HID ** -0.5).astype(np.float32),
        wk=(rng.standard_normal((NKV * HD, HID)) * HID ** -0.5).astype(np.float32),
        wv=(rng.standard_normal((NKV * HD, HID)) * HID ** -0.5).astype(np.float32),
        wo=(rng.standard_normal((HID, HID)) * HID ** -0.5).astype(np.float32),
    )
    out = kernel(**ins)
    print("kernel ran, out shape", out.shape, "mean", float(np.abs(out).mean()))


# revision 29
# speedup vs baseline: 1.4667x; 1.4667x over previous
import sys

sys.path.insert(0, "/opt/trn_rl_repo")

import numpy as np
import ml_dtypes
from contextlib import ExitStack

import concourse.bass as bass
import concourse.tile as tile
from concourse import bacc
from concourse import mybir
from concourse.bass_utils import run_bass_kernel_spmd

B, S, HID = 2, 2048, 1024
NH, NKV, HD = 16, 4, 64
P = 128
NK = HID // P
NQC = S // 512
NST = S // P
QH = NH // NKV
FEAT = QH * HD
MASK_NEG = -1e9

F32 = mybir.dt.float32
F32R = mybir.dt.float32r
BF16 = mybir.dt.bfloat16


def _pin_act_tables():
    import concourse.hw_specs as hw_specs
    import concourse.bacc as bacc_mod
    real = hw_specs.get_activation_tables

    def pinned(arch):
        tabs = dict(real(arch))
        return {name: (funcs if name == "natural_log_exp_and_others" else set())
                for name, funcs in tabs.items()}

    bacc_mod.get_activation_tables = pinned


def build_program():
    _pin_act_tables()
    nc = bacc.Bacc("TRN2", target_bir_lowering=False, debug=False)

    d_xT = nc.dram_tensor("xT", [HID, S], BF16, kind="ExternalInput").ap()
    d_wqT = nc.dram_tensor("wqT", [HID, FEAT], BF16, kind="ExternalInput").ap()
    d_wkvT = nc.dram_tensor("wkvT", [HID, 2 * HD], BF16,
                            kind="ExternalInput").ap()
    d_woT = nc.dram_tensor("woT", [FEAT, HID], BF16, kind="ExternalInput").ap()
    d_cosT = nc.dram_tensor("cosT", [P, S], BF16, kind="ExternalInput").ap()
    d_sinT = nc.dram_tensor("sinT", [P, S], BF16, kind="ExternalInput").ap()
    d_r128 = nc.dram_tensor("r128", [P, P], BF16, kind="ExternalInput").ap()
    d_rdup = nc.dram_tensor("rdup", [HD, P], BF16, kind="ExternalInput").ap()
    d_idup = nc.dram_tensor("idup", [HD, P], BF16, kind="ExternalInput").ap()
    d_ident = nc.dram_tensor("ident", [P, HD], BF16, kind="ExternalInput").ap()
    d_ishift = nc.dram_tensor("ishift", [HD, P], BF16,
                              kind="ExternalInput").ap()
    d_tri = nc.dram_tensor("tri", [P, 2 * P], BF16, kind="ExternalInput").ap()
    d_ones1 = nc.dram_tensor("ones1c", [1, HD], F32R, kind="ExternalInput").ap()
    d_i128b = nc.dram_tensor("i128b", [P, P], BF16, kind="ExternalInput").ap()
    d_out = nc.dram_tensor("outp", [S, HID], BF16, kind="ExternalOutput").ap()

    with tile.TileContext(nc) as tc, ExitStack() as ctx, \
            nc.allow_low_precision(reason="bf16 compute fits 2e-2 tolerance"):
        consts = ctx.enter_context(tc.tile_pool(name="consts", bufs=1))
        main = ctx.enter_context(tc.tile_pool(name="main", bufs=1))

        wq_sb = consts.tile([P, NK, FEAT], BF16)
        nc.scalar.dma_start(wq_sb[:], d_wqT.rearrange("(ko p) m -> p ko m", p=P))
        wkv_sb = consts.tile([P, NK, 2 * HD], BF16)
        nc.scalar.dma_start(wkv_sb[:],
                            d_wkvT.rearrange("(ko p) m -> p ko m", p=P))
        cos_sb = consts.tile([P, S], BF16)
        nc.scalar.dma_start(cos_sb[:], d_cosT)
        r128_sb = consts.tile([P, P], BF16)
        nc.sync.dma_start(r128_sb[:], d_r128)
        rdup_sb = consts.tile([HD, P], BF16)
        nc.sync.dma_start(rdup_sb[:], d_rdup)
        idup_sb = consts.tile([HD, P], BF16)
        nc.sync.dma_start(idup_sb[:], d_idup)

        dma_engs = [nc.sync, nc.gpsimd]
        xsb = [[None] * NK for _ in range(NQC)]

        def _load_x(n_lo, n_hi):
            for n in range(n_lo, n_hi):
                for k in range(NK):
                    xt = main.tile([P, 512], BF16, name=f"x{n}_{k}")
                    eng = dma_engs[(n * NK + k) % 2]
                    eng.dma_start(
                        xt[:],
                        d_xT[k * P:(k + 1) * P, n * 512:(n + 1) * 512])
                    xsb[n][k] = xt

        _load_x(0, 2)
        sin_sb = consts.tile([P, S], BF16)
        nc.gpsimd.dma_start(sin_sb[:], d_sinT)
        _load_x(2, NQC)

        ident_sb = consts.tile([P, HD], BF16)
        nc.sync.dma_start(ident_sb[:], d_ident)
        ishift_sb = consts.tile([HD, P], BF16)
        nc.sync.dma_start(ishift_sb[:], d_ishift)
        tri_sb = consts.tile([P, 2, P], BF16)
        nc.sync.dma_start(tri_sb[:], d_tri)
        i128b_sb = consts.tile([P, P], BF16)
        nc.sync.dma_start(i128b_sb[:], d_i128b)
        ones1 = consts.tile([1, HD], F32R)
        nc.sync.dma_start(ones1[:], d_ones1)
        wo_sb = consts.tile([P, 2, HID], BF16)
        nc.gpsimd.dma_start(wo_sb[:], d_woT.rearrange("(ko p) m -> p ko m", p=P))

        qpt = main.tile([P, 2, S], BF16)
        kpt = main.tile([P, S], BF16)
        vaug = main.tile([P, NST, HD + 1], BF16)
        attnT = main.tile([P, 2, S], BF16)

        nc.vector.memset(vaug[:, :, HD:HD + 1], 1.0)

        scps = ctx.enter_context(tc.tile_pool(name="scps", bufs=2, space="PSUM"))
        pvps = ctx.enter_context(tc.tile_pool(name="pvps", bufs=1, space="PSUM"))
        util = ctx.enter_context(tc.tile_pool(name="util", bufs=2, space="PSUM"))

        rawp = ctx.enter_context(tc.tile_pool(name="rawp", bufs=3))
        tmpp = ctx.enter_context(tc.tile_pool(name="tmpp", bufs=2))
        ptp = ctx.enter_context(tc.tile_pool(name="ptp", bufs=4))
        recp = ctx.enter_context(tc.tile_pool(name="recp", bufs=2))
        outp = ctx.enter_context(tc.tile_pool(name="outp", bufs=3))

        for i in range(32):
            wp = util.tile([P, P], F32, tag="ut", name=f"warm{i}")
            nc.tensor.matmul(wp[:], r128_sb[:], r128_sb[:],
                             start=True, stop=True)


        def emit_proj_q(n, m2):
            c0 = n * 512
            ps = util.tile([P, 512], F32, tag="ut", name=f"pjq{n}{m2}")
            for k in range(NK):
                nc.tensor.matmul(ps[:], wq_sb[:, k, m2 * P:(m2 + 1) * P],
                                 xsb[n][k][:],
                                 start=(k == 0), stop=(k == NK - 1))
            raw = rawp.tile([P, 512], BF16, tag="raw", name=f"qraw{n}{m2}")
            nc.vector.tensor_copy(raw[:], ps[:])
            psr = util.tile([P, 512], F32, tag="ut", name=f"pjqr{n}{m2}")
            nc.tensor.matmul(psr[:], r128_sb[:], raw[:], start=True, stop=True)
            t1 = tmpp.tile([P, 512], BF16, tag="t1", name=f"t1q{n}{m2}")
            nc.vector.tensor_mul(t1[:], raw[:], cos_sb[:, c0:c0 + 512])
            t2 = tmpp.tile([P, 512], BF16, tag="t2", name=f"t2q{n}{m2}")
            nc.vector.tensor_mul(t2[:], psr[:], sin_sb[:, c0:c0 + 512])
            nc.vector.tensor_add(qpt[:, m2, c0:c0 + 512], t1[:], t2[:])

        def emit_proj_kv(n):
            c0 = n * 512
            ps = util.tile([P, 512], F32, tag="ut", name=f"pjkv{n}")
            for k in range(NK):
                nc.tensor.matmul(ps[:], wkv_sb[:, k, :], xsb[n][k][:],
                                 start=(k == 0), stop=(k == NK - 1))
            raw = rawp.tile([P, 512], BF16, tag="raw", name=f"kvraw{n}")
            nc.vector.tensor_copy(raw[:], ps[:])
            psk2 = util.tile([P, 512], F32, tag="ut", name=f"pjk2{n}")
            nc.tensor.matmul(psk2[:], idup_sb[:], raw[0:HD, :],
                             start=True, stop=True)
            t1 = tmpp.tile([P, 512], BF16, tag="t1", name=f"t1k{n}")
            nc.vector.tensor_mul(t1[:], psk2[:], cos_sb[:, c0:c0 + 512])
            pskr = util.tile([P, 512], F32, tag="ut", name=f"pjkr{n}")
            nc.tensor.matmul(pskr[:], rdup_sb[:], raw[0:HD, :],
                             start=True, stop=True)
            t2 = tmpp.tile([P, 512], BF16, tag="t2", name=f"t2k{n}")
            nc.vector.tensor_mul(t2[:], pskr[:], sin_sb[:, c0:c0 + 512])
            nc.vector.tensor_add(kpt[:, c0:c0 + 512], t1[:], t2[:])
            for tt in range(4):
                sq = 4 * n + tt
                psv = util.tile([P, HD], BF16, tag="ut", name=f"vt{n}{tt}")
                nc.tensor.transpose(psv[:], raw[HD:P, tt * P:(tt + 1) * P],
                                    ident_sb[HD:P, :])
                nc.vector.tensor_copy(vaug[:, sq, 0:HD], psv[:])

        def emit_outproj(st_, nn):
            po = util.tile([P, 512], F32, tag="ut", name=f"po{st_}{nn}")
            for m in range(2):
                nc.tensor.matmul(po[:], attnT[:, m, st_ * P:(st_ + 1) * P],
                                 wo_sb[:, m, nn * 512:(nn + 1) * 512],
                                 start=(m == 0), stop=(m == 1))
            ot = outp.tile([P, 512], BF16, tag="ot", name=f"ot{st_}{nn}")
            if (st_ * 2 + nn) % 2 == 1:
                nc.scalar.activation(ot[:], po[:],
                                     mybir.ActivationFunctionType.Copy)
            else:
                nc.vector.tensor_copy(ot[:], po[:])
            dma_engs[(st_ * 2 + nn) % 2].dma_start(
                d_out[st_ * P:(st_ + 1) * P, nn * 512:(nn + 1) * 512], ot[:])

        fillers = []
        emitted = set()

        def pop_filler():
            if fillers:
                nm, fn = fillers.pop(0)
                fn()
                if nm:
                    emitted.add(nm)

        def drain_until(name):
            if name in emitted:
                return
            while fillers:
                nm, fn = fillers.pop(0)
                fn()
                if nm:
                    emitted.add(nm)
                if nm == name:
                    return

        def emit_attn(m, j):
            c0 = j * 512
            T = 4 * j + 4
            pv = pvps.tile([HD + 1, 2, 512], F32, tag="pv", name=f"pv{m}{j}")

            def emit_sc(t):
                r = t - 4 * j
                lo = P * r if r >= 0 else 0
                sc = scps.tile([P, 2, 512], F32, tag="sc", name=f"sc{m}{j}{t}")
                for h2 in (0, 1):
                    half = h2 * HD
                    kl = kpt[half:half + HD, t * P:(t + 1) * P]
                    ql = qpt[half:half + HD, m, c0 + lo:c0 + 512]
                    if r >= 0:
                        nc.tensor.matmul(sc[:, h2, lo:512], kl, ql,
                                         start=True, stop=False,
                                         skip_group_check=True)
                        nc.tensor.matmul(sc[:, h2, lo:lo + P], i128b_sb[:],
                                         tri_sb[:, h2, :], start=False,
                                         stop=True, skip_group_check=True)
                    else:
                        nc.tensor.matmul(sc[:, h2, :], kl, ql,
                                         start=True, stop=True,
                                         skip_group_check=True)
                return sc, lo

            cur = emit_sc(0)
            for t in range(T):
                sc, lo = cur
                pt = ptp.tile([P, 2, 512], BF16, tag="pt", name=f"pt{m}{j}{t}")
                nc.scalar.activation(pt[:], sc[:],
                                     mybir.ActivationFunctionType.Exp,
                                     scale=0.125)
                pop_filler()
                if t + 1 < T:
                    cur = emit_sc(t + 1)
                for h2 in (0, 1):
                    nc.tensor.matmul(pv[0:HD + 1, h2, lo:512],
                                     vaug[:, t, :], pt[:, h2, lo:512],
                                     start=(t == 0), stop=(t == T - 1),
                                     skip_group_check=True)
            pvc = recp.tile([HD + 1, 2, 512], BF16, tag="pvc",
                            name=f"pvc{m}{j}")
            nc.vector.tensor_copy(pvc[:], pv[:])
            lnt = recp.tile([1, 2, 512], F32, tag="lnt", name=f"lnt{m}{j}")
            nc.scalar.activation(lnt[:], pvc[HD:HD + 1, :, :],
                                 mybir.ActivationFunctionType.Ln)
            recr = recp.tile([1, 2, 512], F32R, tag="recr", name=f"recr{m}{j}")
            nc.scalar.activation(recr[:], lnt[:],
                                 mybir.ActivationFunctionType.Exp, scale=-1.0)

            pop_filler()
            for h2 in (0, 1):
                rec_b = util.tile([HD, 512], F32, tag="ut",
                                  name=f"rb{m}{j}{h2}")
                nc.tensor.matmul(rec_b[:], ones1[:], recr[:, h2, :],
                                 start=True, stop=True)
                rec_s = recp.tile([HD, 512], BF16, tag="recs",
                                  name=f"rcs{m}{j}{h2}")
                nc.vector.tensor_copy(rec_s[:], rec_b[:])
                if h2 == 0:
                    nc.vector.tensor_mul(attnT[0:HD, m, c0:c0 + 512],
                                         pvc[0:HD, 0, :], rec_s[:])
                else:
                    oddt = recp.tile([HD, 512], BF16, tag="oddt",
                                     name=f"odd{m}{j}")
                    nc.vector.tensor_mul(oddt[:], pvc[0:HD, 1, :], rec_s[:])
                    rp = util.tile([P, 512], F32, tag="ut", name=f"rp{m}{j}")
                    nc.tensor.matmul(rp[:], ishift_sb[:], oddt[:],
                                     start=True, stop=True)
                    nc.vector.tensor_copy(attnT[HD:P, m, c0:c0 + 512],
                                          rp[HD:P, :])

        emit_proj_q(0, 0)
        emit_proj_kv(0)
        fillers.extend([
            ("q01", lambda: emit_proj_q(0, 1)),
            ("q10", lambda: emit_proj_q(1, 0)),
            ("kv1", lambda: emit_proj_kv(1)),
            ("q11", lambda: emit_proj_q(1, 1)),
            ("q20", lambda: emit_proj_q(2, 0)),
            ("kv2", lambda: emit_proj_kv(2)),
            ("q21", lambda: emit_proj_q(2, 1)),
            ("q30", lambda: emit_proj_q(3, 0)),
            ("kv3", lambda: emit_proj_kv(3)),
            ("q31", lambda: emit_proj_q(3, 1)),
        ])

        def add_outproj(st_lo, st_hi):
            for st_ in range(st_lo, st_hi):
                for nn in range(2):
                    fillers.append(
                        (f"op{st_}{nn}",
                         lambda st_=st_, nn=nn: emit_outproj(st_, nn)))

        emit_attn(0, 0)
        drain_until("q01")
        emit_attn(1, 0)
        add_outproj(0, 2)
        drain_until("kv1")
        drain_until("q10")
        emit_attn(0, 1)
        add_outproj(2, 4)
        drain_until("q11")
        emit_attn(1, 1)
        add_outproj(4, 6)
        drain_until("kv2")
        drain_until("q20")
        emit_attn(0, 2)
        add_outproj(6, 8)
        drain_until("q21")
        emit_attn(1, 2)
        add_outproj(8, 10)
        drain_until("kv3")
        drain_until("q30")
        emit_attn(0, 3)
        add_outproj(10, 12)
        drain_until("q31")
        emit_attn(1, 3)
        add_outproj(12, NST)
        while fillers:
            fillers.pop(0)[1]()

    nc.compile()
    return nc


def make_consts():
    bf = ml_dtypes.bfloat16
    r128 = np.zeros((P, P), np.float32)
    for mm in range(P):
        hh, dd = mm // HD, mm % HD
        if dd < HD // 2:
            r128[hh * HD + dd + HD // 2, mm] = -1.0
        else:
            r128[hh * HD + dd - HD // 2, mm] = 1.0
    rdup = np.zeros((HD, P), np.float32)
    idup = np.zeros((HD, P), np.float32)
    for mm in range(P):
        dd = mm % HD
        idup[dd, mm] = 1.0
        if dd < HD // 2:
            rdup[dd + HD // 2, mm] = -1.0
        else:
            rdup[dd - HD // 2, mm] = 1.0
    ident = np.zeros((P, HD), np.float32)
    ident[HD:P, :] = np.eye(HD)
    ishift = np.zeros((HD, P), np.float32)
    for kk in range(HD):
        ishift[kk, kk + HD] = 1.0
    tri1 = np.where(np.arange(P)[:, None] <= np.arange(P)[None, :], 0.0,
                    MASK_NEG).astype(bf)
    tri = np.concatenate([tri1, tri1], axis=1)
    i128b = np.eye(P).astype(bf)
    return dict(r128=r128.astype(bf), rdup=rdup.astype(bf),
                idup=idup.astype(bf), ident=ident.astype(bf),
                ishift=ishift.astype(bf), tri=tri, i128b=i128b,
                ones1c=np.ones((1, HD), np.float32))


def make_in_maps(x, cos, sin, wq, wk, wv, wo):
    bf = ml_dtypes.bfloat16
    consts = make_consts()
    cosT = np.ascontiguousarray(np.vstack([cos.T, cos.T])).astype(bf)
    sinT = np.ascontiguousarray(np.vstack([sin.T, sin.T])).astype(bf)
    in_maps = []
    for core in range(8):
        b, g = core // NKV, core % NKV
        xT = np.ascontiguousarray(x[b].T).astype(bf)
        wqT = np.ascontiguousarray(wq[g * FEAT:(g + 1) * FEAT, :].T).astype(bf)
        wkvT = np.ascontiguousarray(
            np.concatenate([wk[g * HD:(g + 1) * HD, :],
                            wv[g * HD:(g + 1) * HD, :]], axis=0).T).astype(bf)
        woT = np.ascontiguousarray(wo[:, g * FEAT:(g + 1) * FEAT].T).astype(bf)
        in_maps.append(dict(xT=xT, wqT=wqT, wkvT=wkvT, woT=woT,
                            cosT=cosT, sinT=sinT, **consts))
    return in_maps


_PROG = None


def kernel(x, cos, sin, wq, wk, wv, wo):
    global _PROG
    x = np.asarray(x, np.float32)
    cos = np.asarray(cos, np.float32)
    sin = np.asarray(sin, np.float32)
    wq = np.asarray(wq, np.float32)
    wk = np.asarray(wk, np.float32)
    wv = np.asarray(wv, np.float32)
    wo = np.asarray(wo, np.float32)

    in_maps = make_in_maps(x, cos, sin, wq, wk, wv, wo)
    if _PROG is None:
        _PROG = build_program()
    res = run_bass_kernel_spmd(_PROG, in_maps, core_ids=list(range(8)))

    out = np.zeros((B, S, HID), np.float32)
    for core in range(8):
        out[core // NKV] += np.asarray(res.results[core]["outp"], np.float32)
    return out


if __name__ == "__main__":
    rng = np.random.default_rng(0)
    ins = dict(
        x=rng.standard_normal((B, S, HID)).astype(np.float32),
        cos=rng.random((S, HD)).astype(np.float32),
        sin=rng.random((S, HD)).astype(np.float32),
        wq=(rng.standard_normal((HID, HID)) * HID ** -0.5).astype(np.float32),
        wk=(rng.standard_normal((NKV * HD, HID)) * HID ** -0.5).astype(np.float32),
        wv=(rng.standard_normal((NKV * HD, HID)) * HID ** -0.5).astype(np.float32),
        wo=(rng.standard_normal((HID, HID)) * HID ** -0.5).astype(np.float32),
    )
    out = kernel(**ins)
    print("kernel ran, out shape", out.shape, "mean", float(np.abs(out).mean()))
